# revision 1
# baseline (speedup 1.0000x reference)
"""HSIC loss kernel for Trainium2, 8 NeuronCores.

Math: for each feature column c of X [2048, 16], K_c = rbf kernel matrix
(zero diag). Output = sum over feature pairs a<b of squared unbiased-HSIC
combination of T[a,b]=sum(K_a*K_b), rowsums A, total sums S.

Device strategy (SPMD, symmetric-triangle sharding, all-f32 precision):
  The kernel matrix is symmetric, so each unordered 128x128 block pair is
  computed once: j-block jb stores columns i in [128*jb, 128*jb + span)
  (mod 2048), span = 9 blocks for jb<8, 8 blocks for jb>=8 -- exact single
  coverage.  Core r owns jb=r (span 1152) and jb=r+8 (span 1024); the host
  rotates each core's xrep planes by 128*r so every core compiles the SAME
  instruction stream.
     K' = Derivative_Erf(sqrt(g_c)*xi - sqrt(g_c)*xj) = (2/sqrt(pi)) * rbf
  One ACT per (block, c) with the block span as free dim, f32r output into a
  blocked [(i//8), c, i%8] ktile (f32r matmuls keep full f32 precision at
  1 cycle/row for moving>=256, so NOTHING is ever rounded below f32 and the
  huge T-vs-A cancellation in the HSIC estimator stays exact).  accum_out
  gives stored-row sums; PE ones-matmuls give per-block column sums; host
  assembles full rowsums A from both (symmetry), T from diag/off-diag
  even/odd PSUM gram accumulators, all in float64.
"""

import sys
import numpy as np

if "/opt/trn_rl_repo" not in sys.path:
    sys.path.insert(0, "/opt/trn_rl_repo")

N = 2048
D = 16
P = 128
NCORES = 8
GJ = 8                    # i's packed per gram matmul -> D*GJ = 128 cols
CH = D * GJ               # 128
S0, S1 = 1152, 1024       # spans of block 0 (jb=r) and block 1 (jb=r+8)
OFF1 = 1024               # plane offset of block 1 (= 128*(r+8) - 128*r)
NCS0, NCS1 = S0 * D // 512, S1 * D // 512   # 36, 32 colsum matmuls
SQPI = float(np.sqrt(np.pi))
C32 = np.float32(2.0 / np.sqrt(np.pi))      # DErf(0) table value on device

_NC_CACHE = {}


def _patch_tile_drain():
    """Walrus in this container accepts only 1 sync-wait per instruction.
    Tile routinely attaches several. Hoist extra waits onto single-wait NoOp
    carriers emitted just before the instruction on the same engine, and
    split the tail drain's per-engine waits the same way."""
    import concourse.mybir as mybir
    import concourse.tile as tile_mod
    from concourse.vector_clock import ScopedClock, VectorClock

    if getattr(tile_mod.TileContext, "_drain_patched", False):
        return

    orig_add = tile_mod.TileContext._add_instruction
    counter = [0]

    def _add_instruction(self, inst):
        si = inst.sync_info
        if si is not None and si.on_wait is not None and len(si.on_wait) > 1:
            waits = list(si.on_wait)
            for w in waits[:-1]:
                counter[0] += 1
                carrier = mybir.InstNoOp(name=f"waitc-{counter[0]}")
                carrier.engine = inst.engine
                carrier.sync_info = mybir.SyncInfo(on_wait=[w], on_update=[])
                orig_add(self, carrier)
            inst.sync_info = mybir.SyncInfo(
                on_wait=[waits[-1]], on_update=list(si.on_update or [])
            )
        orig_add(self, inst)

    def _drain_and_barrier(self, tick_clock, wait_clock):
        vec = list(tick_clock.global_clock)
        for i, v in enumerate(vec):
            if v <= 0:
                continue
            sub = [v if j == i else 0 for j in range(len(vec))]
            carrier = self.nc.sync.nop(nofuse=True)
            wait_clock.add_sem_waits(
                carrier.ins, ScopedClock({None: VectorClock(sub)})
            )
        self.nc.sync.drain()
        self.nc.all_engine_barrier()
        popped = self.nc._tile_sem_poison_stack.pop()
        assert popped is self._sem_poison
        self.nc.clear_and_free_semaphores(list(self.sems.allocated().values()))
        self.nc.all_engine_barrier()

    tile_mod.TileContext._add_instruction = _add_instruction
    tile_mod.TileContext._drain_and_barrier = _drain_and_barrier
    tile_mod.TileContext._drain_patched = True


def _build_nc():
    import concourse.bass as bass
    import concourse.mybir as mybir
    from concourse.tile import TileContext

    _patch_tile_drain()

    f32 = mybir.dt.float32
    f32r = mybir.dt.float32r
    f16 = mybir.dt.float16

    nc = bass.Bass("TRN2")
    xrep_d = nc.dram_tensor("xrep", [P, D * N], f16, kind="ExternalInput")
    scale_d = nc.dram_tensor("scale_t", [P, D], f32, kind="ExternalInput")
    bias_d = nc.dram_tensor("bias_t", [P, 2 * D], f32, kind="ExternalInput")
    ones_d = nc.dram_tensor("ones_i", [P, 1], f32r, kind="ExternalInput")
    apart_d = nc.dram_tensor("apart", [P, 4 * D], f32, kind="ExternalOutput")
    gde_d = nc.dram_tensor("gde", [P, 256], f32, kind="ExternalOutput")
    gdo_d = nc.dram_tensor("gdo", [P, 256], f32, kind="ExternalOutput")
    goe_d = nc.dram_tensor("goe", [P, 256], f32, kind="ExternalOutput")
    goo_d = nc.dram_tensor("goo", [P, 256], f32, kind="ExternalOutput")
    cs_d = nc.dram_tensor("cs", [8, 9 * 512], f32, kind="ExternalOutput")

    # per-psum-tile matmul counts for start/stop flags
    def uses(span):
        nch = span // GJ
        return np.array(
            [
                sum(1 for g in range(nch) if g < 16 and g % 2 == 0),
                sum(1 for g in range(nch) if g < 16 and g % 2 == 1),
                sum(1 for g in range(nch) if g >= 16 and g % 2 == 0),
                sum(1 for g in range(nch) if g >= 16 and g % 2 == 1),
            ]
        )

    total_uses = uses(S0) + uses(S1)

    with TileContext(nc) as tc:
        with (
            tc.tile_pool(name="const", bufs=1) as cpool,
            tc.tile_pool(name="xp", bufs=12) as xpool,
            tc.tile_pool(name="st", bufs=1) as stpool,
            tc.tile_pool(name="cstr", bufs=2) as strpool,
            tc.tile_pool(name="gps", bufs=1, space="PSUM") as gpool,
            tc.tile_pool(name="cps", bufs=4, space="PSUM") as cspool,
        ):
            scale_sb = cpool.tile([P, D], f32)
            bias_sb = cpool.tile([P, 2 * D], f32)
            ones = cpool.tile([P, 1], f32r)
            abuf = cpool.tile([P, 4 * D], f32)
            kt0 = cpool.tile([P, S0 * D], f32r)
            kt1 = cpool.tile([P, S1 * D], f32r)

            nc.sync.dma_start(scale_sb[:], scale_d[:])
            nc.sync.dma_start(bias_sb[:], bias_d[:])
            nc.sync.dma_start(ones[:], ones_d[:])

            grams = [gpool.tile([P, 256], f32, name=f"gram{t}") for t in range(4)]
            use_ct = [0, 0, 0, 0]
            cs_row = 0

            for blk, (kt, S, off) in enumerate(((kt0, S0, 0), (kt1, S1, OFF1))):
                k4 = kt[:].rearrange("p (g c i) -> p g c i", c=D, i=GJ)
                Sh = S // 2          # i-span per ACT phase
                Gh = Sh // GJ        # chunks per phase
                for ph in range(2):
                    for c in range(D):
                        xpl = xpool.tile([P, Sh], f16, name=f"xpl{blk}")
                        base = c * N + off + ph * Sh
                        nc.sync.dma_start(xpl[:], xrep_d[:, base : base + Sh])
                        col = (blk * 2 + ph) * D + c
                        nc.scalar.activation(
                            out=k4[:, ph * Gh : (ph + 1) * Gh, c, :],
                            in_=xpl[:],
                            func=mybir.ActivationFunctionType.Derivative_Erf,
                            bias=bias_sb[:, blk * D + c : blk * D + c + 1],
                            scale=scale_sb[:, c : c + 1],
                            accum_out=abuf[:, col : col + 1],
                        )
                    # grams + colsums for this phase, interleaved 4:1 so the
                    # colsum psum drain (DVE copies) never stalls the PE;
                    # copies land in a half-phase strip, one DMA per strip
                    nh = Gh // 8                 # colsums per half-phase (9/8)
                    mlist = list(range(ph * Gh // 4, (ph + 1) * Gh // 4))
                    strip = None
                    for k, m in enumerate(mlist):
                        for g in range(4 * m, 4 * m + 4):
                            lhsT = kt[:, g * CH : (g + 1) * CH]
                            q = g // 2
                            rhs = kt[:, q * 256 : (q + 1) * 256]
                            t = (0 if g < 16 else 2) + (g % 2)
                            nc.tensor.matmul(
                                grams[t][:],
                                lhsT=lhsT,
                                rhs=rhs,
                                start=(use_ct[t] == 0),
                                stop=(use_ct[t] == total_uses[t] - 1),
                            )
                            use_ct[t] += 1
                        cp = cspool.tile([1, 512], f32, name="cp")
                        nc.tensor.matmul(
                            cp[:],
                            lhsT=ones[:],
                            rhs=kt[:, m * 512 : (m + 1) * 512],
                            start=True,
                            stop=True,
                        )
                        if strip is None:
                            strip = strpool.tile([1, 9 * 512], f32, name="csstrip")
                        seg = k % nh
                        nc.vector.tensor_copy(
                            strip[:, seg * 512 : (seg + 1) * 512], cp[:]
                        )
                        if seg == nh - 1:
                            nc.gpsimd.dma_start(
                                cs_d[cs_row : cs_row + 1, : nh * 512],
                                strip[:, : nh * 512],
                            )
                            cs_row += 1
                            strip = None

            nc.gpsimd.dma_start(apart_d[:], abuf[:])
            for t, dst in enumerate((gde_d, gdo_d, goe_d, goo_d)):
                gst = stpool.tile([P, 256], f32, name=f"gst{t}")
                nc.vector.tensor_copy(gst[:], grams[t][:])
                nc.gpsimd.dma_start(dst[:], gst[:])
    return nc


def _get_nc():
    if "nc" not in _NC_CACHE:
        _NC_CACHE["nc"] = _build_nc()
    return _NC_CACHE["nc"]


def _make_in_maps(X):
    Xd = X.astype(np.float64)
    meanD = 2.0 * (np.mean(Xd * Xd, axis=0) - np.mean(Xd, axis=0) ** 2)  # [D]
    g = 1.0 / (2.0 * meanD)                # gamma = 1/(2*sigma^2)
    s = np.sqrt(g).astype(np.float32)      # sqrt(gamma) per column

    # Device sees fp16-rounded samples; build the bias from the same rounded
    # values so the kernel diagonal is DErf(0) exactly.
    X16 = X.astype(np.float16)
    scale_t = np.ascontiguousarray(np.broadcast_to(s[None, :], (P, D)))
    ones_i = np.ones((P, 1), np.float32)

    in_maps = []
    for r in range(NCORES):
        # xrep plane i' = X16[(128*r + i') mod 2048, c], broadcast on partitions
        rot = np.roll(X16, -128 * r, axis=0)          # [N, D] f16
        xrep = np.ascontiguousarray(
            np.broadcast_to(rot.T.reshape(1, D * N), (P, D * N))
        )
        # bias rows: block 0 -> j in jb=r, block 1 -> j in jb=r+8
        bias = np.empty((P, 2 * D), np.float32)
        for blk, jb in enumerate((r, r + 8)):
            xj = X16[jb * P : (jb + 1) * P, :].astype(np.float32)  # [P, D]
            bias[:, blk * D : (blk + 1) * D] = -(s[None, :] * xj)
        in_maps.append(
            {
                "xrep": xrep,
                "scale_t": scale_t.astype(np.float32),
                "bias_t": bias,
                "ones_i": ones_i,
            }
        )
    return in_maps


def _combine(results):
    # Device K' = (2/sqrt(pi)) * K, exact f32 everywhere.
    Ap = np.zeros((D, N), dtype=np.float64)   # full rowsums of K' incl diag
    Tp = np.zeros((D, D), dtype=np.float64)
    cdiag = float(C32)
    for r in range(NCORES):
        res = results[r]
        ap4 = res["apart"].astype(np.float64)         # [P, 4D] (blk, phase, c)
        ap = ap4.reshape(P, 2, 2, D).sum(axis=2).reshape(P, 2 * D)
        cs2 = res["cs"].astype(np.float64)            # [8, 9*512] strips
        rows = []
        hi = 0
        for nh in (9, 9, 9, 9, 8, 8, 8, 8):
            rows.append(cs2[hi, : nh * 512].reshape(nh, 512))
            hi += 1
        cs = np.concatenate(rows)                     # [68, 512]
        cs_row = 0
        for blk, (jb, S) in enumerate(((r, S0), (r + 8, S1))):
            # stored-row sums: rows j in jb-block over the block's i-span
            for c in range(D):
                Ap[c, jb * P : (jb + 1) * P] += ap[:, blk * D + c]
            # column sums: cols are the block's local i (skip own diag block)
            ncs = S * D // 512
            blkcs = cs[cs_row : cs_row + ncs].reshape(ncs * 4, D, GJ)
            cs_row += ncs
            # chunk g covers local i in [g*8, g*8+8)
            for gl in range(ncs * 4):
                if gl < 16:
                    continue  # own diag block: already in accum rows
                i0 = (128 * jb + gl * GJ) % N
                Ap[:, i0 : i0 + GJ] += blkcs[gl]
        gde = res["gde"].astype(np.float64)[:, :128]
        gdo = res["gdo"].astype(np.float64)[:, 128:]
        goe = res["goe"].astype(np.float64)[:, :128]
        goo = res["goo"].astype(np.float64)[:, 128:]
        diag = (gde + gdo).reshape(D, GJ, D, GJ)
        offd = (goe + goo).reshape(D, GJ, D, GJ)
        Tp += np.einsum("aibi->ab", diag) + 2.0 * np.einsum("aibi->ab", offd)

    A = (SQPI / 2.0) * (Ap - cdiag)             # undo 2/sqrt(pi), remove diag
    T = (np.pi / 4.0) * (Tp - N * cdiag * cdiag)
    S = A.sum(axis=1)
    Dm = A @ A.T
    c0 = 1.0 / (N * (N - 3))
    hsic = c0 * (
        T + np.outer(S, S) / ((N - 1.0) * (N - 2.0)) - (2.0 / (N - 2.0)) * Dm
    )
    iu = np.triu_indices(D, 1)
    return np.float32(np.sum(hsic[iu] ** 2))


def run_spmd(in_maps, **kwargs):
    from concourse import bass_utils

    nc = _get_nc()
    return bass_utils.run_bass_kernel_spmd(
        nc, in_maps, core_ids=list(range(NCORES)), **kwargs
    )


def kernel(X):
    X = np.ascontiguousarray(np.asarray(X, dtype=np.float32))
    in_maps = _make_in_maps(X)
    res = run_spmd(in_maps)
    return _combine(res.results)

